# revision 4
# baseline (speedup 1.0000x reference)
"""Trainium2 Bass kernel for nn_CrossVariableMixingConv.

Reference computation (per row of x, B*L rows, C=862 channels):
    h   = conv1d(x, Wup, k=7, pad=3) + bup      # (RANK=8 channels)
    g   = gelu(h)  (erf-exact)
    d   = sum_r Wdown[r] * g[r] + bdown
    y   = LayerNorm(x + d) * gamma + beta       # LN over C

Sharding: pure data parallelism — the B*L = 11520 rows are split into 8
shards of 1440 rows, one per NeuronCore; the ~70 conv params are
replicated.

Per-core kernel structure (rows on partitions, chunks of 128 rows):
  - The k=7 conv is a matmul with the *data* as the stationary operand:
    lhsT = xT window [128 c-taps, n rows] (DMA-transposed load of a
    pre-windowed bf16 copy of x, with a baked-in ones column), rhs = a
    banded weight matrix [128, 484] covering 4 ranks x 121 output
    columns; the ones row x bias row adds bup inside the same matmul.
  - Gelu runs on the Scalar engine directly out of PSUM (erf-exact Gelu
    LUT), writing bf16 into a per-rank G buffer in SBUF.
  - The rank contraction (Wdown) is 8 accumulating scaled-identity
    matmuls into PSUM, which leaves d already in row-major layout.
  - Residual + LayerNorm stats (bn_stats/bn_aggr) run on the Vector
    engine per chunk; rstd = 1/sqrt(var+eps) is batched once per core
    (single ACT table switch), then the normalization is applied with a
    fused (y - mu) * rstd tensor_scalar and results are DMA'd out.

bdown is dropped: LayerNorm is invariant to a constant shift per row.
gamma/beta are applied only when they are not the identity (they are
ones/zeros in this problem's setup_inputs).
"""

import sys

for _p in ("/opt/trn_rl_repo",):
    if _p not in sys.path:
        sys.path.insert(0, _p)

import numpy as np
import ml_dtypes

B, L, C = 16, 720, 862
RANK, KTAPS = 8, 7
NCORES = 8
ROWS = B * L                 # 11520
RPC = ROWS // NCORES         # 1440 rows per core
PCH = 128                    # rows per chunk (partition dim)
NCHUNK = (RPC + PCH - 1) // PCH   # 12 (11 full + 1 of 32)
WW = 121                     # conv output columns per window
NW = 8                       # windows: 8*121 = 968 >= 862
WIN = 128                    # window input columns (127 data + 1 ones)
GW = NW * WW                 # 968
XPADC = 976                  # padded x columns: 3 zeros + 862 + tail zeros
CHALF = C // 2               # 431
EPS = 1e-5

_CACHE: dict = {}


def _build(apply_gamma_beta: bool):
    """Build + compile the per-core Bass program. Cached per flag."""
    key = ("nc", apply_gamma_beta)
    if key in _CACHE:
        return _CACHE[key]

    from contextlib import ExitStack

    import concourse.bacc as bacc
    import concourse.bass as bass
    import concourse.tile as tile
    from concourse import mybir

    f32 = mybir.dt.float32
    bf16 = mybir.dt.bfloat16
    AF = mybir.ActivationFunctionType
    ALU = mybir.AluOpType

    nc = bacc.Bacc(
        "TRN2", target_bir_lowering=False, debug=False, num_devices=NCORES
    )

    x_d = nc.dram_tensor("x", [RPC, C], f32, kind="ExternalInput").ap()
    xw_d = nc.dram_tensor("xw", [RPC, NW, WIN], bf16, kind="ExternalInput").ap()
    band_d = nc.dram_tensor("band", [WIN, 2, 4 * WW], bf16, kind="ExternalInput").ap()
    wdi_d = nc.dram_tensor("wdi", [128, RANK, 128], bf16, kind="ExternalInput").ap()
    if apply_gamma_beta:
        gb_d = nc.dram_tensor("gb", [2, C], f32, kind="ExternalInput").ap()
    y_d = nc.dram_tensor("y", [RPC, C], f32, kind="ExternalOutput").ap()

    with tile.TileContext(nc) as tc, ExitStack() as ctx:
        singles = ctx.enter_context(tc.tile_pool(name="singles", bufs=1))
        xp = ctx.enter_context(tc.tile_pool(name="xin", bufs=2))
        xtp = ctx.enter_context(tc.tile_pool(name="xt", bufs=3))
        gp = ctx.enter_context(tc.tile_pool(name="g", bufs=2))
        op = ctx.enter_context(tc.tile_pool(name="o", bufs=2))
        stp = ctx.enter_context(tc.tile_pool(name="st", bufs=2))
        pp = ctx.enter_context(tc.tile_pool(name="ps", bufs=2, space="PSUM"))

        band_t = singles.tile([WIN, 2, 4 * WW], bf16)
        nc.sync.dma_start(out=band_t, in_=band_d)
        wdi_t = singles.tile([128, RANK, 128], bf16)
        nc.sync.dma_start(out=wdi_t, in_=wdi_d)
        if apply_gamma_beta:
            gamma_rep = singles.tile([128, C], f32)
            beta_rep = singles.tile([128, C], f32)
            for rep, row in ((gamma_rep, 0), (beta_rep, 1)):
                src = bass.AP(
                    tensor=gb_d.tensor,
                    offset=gb_d.offset + row * C,
                    ap=[[0, 128], [1, C]],
                )
                nc.gpsimd.dma_start(out=rep, in_=src)

        y_all = singles.tile([128, NCHUNK, C], f32)
        mv_all = singles.tile([128, NCHUNK, 2], f32)
        # The ragged last chunk writes only 32 partitions; pre-fill so the
        # batched rstd (which reads the full tile) sees benign values.
        nc.vector.memset(mv_all, 1.0)

        for ic in range(NCHUNK):
            n0 = ic * PCH
            nr = min(PCH, RPC - n0)

            x_t = xp.tile([128, C], f32)
            nc.sync.dma_start(out=x_t[:nr], in_=x_d[n0 : n0 + nr, :])

            G = gp.tile([128, RANK, GW], bf16)
            # G viewed as [p, w, rh, r4, c] for the gelu writes
            g_wview = G.rearrange("p (rh r4) (w c) -> p w rh r4 c", rh=2, c=WW)

            for g2 in range(4):  # groups of 2 windows -> 4 PSUM banks
                H = pp.tile([128, 4, 512], f32, tag="ps")
                for wg in range(2):
                    w = g2 * 2 + wg
                    xt = xtp.tile([WIN, PCH], bf16)
                    nc.sync.dma_start_transpose(
                        out=xt[:, :nr], in_=xw_d[n0 : n0 + nr, w, :]
                    )
                    for rh in range(2):
                        nc.tensor.matmul(
                            H[:nr, wg * 2 + rh, 0 : 4 * WW],
                            lhsT=xt[:, :nr],
                            rhs=band_t[:, rh, :],
                            start=True,
                            stop=True,
                        )
                h_view = H[:nr, :, 0 : 4 * WW].rearrange(
                    "p (wg rh) (r4 c) -> p wg rh r4 c", rh=2, c=WW
                )
                g_view = g_wview[:nr, g2 * 2 : g2 * 2 + 2]
                nc.scalar.activation(out=g_view, in_=h_view, func=AF.Gelu)

            dT = pp.tile([128, 2, 512], f32, tag="ps")
            for h in range(2):
                for r in range(RANK):
                    nc.tensor.matmul(
                        dT[:nr, h, 0:CHALF],
                        lhsT=wdi_t[:nr, r, :nr],
                        rhs=G[:nr, r, h * CHALF : (h + 1) * CHALF],
                        start=(r == 0),
                        stop=(r == RANK - 1),
                    )

            yc = y_all[:nr, ic, :].rearrange("p (s c) -> p s c", s=2)
            nc.vector.tensor_add(
                out=yc,
                in0=x_t[:nr].rearrange("p (s c) -> p s c", s=2),
                in1=dT[:nr, :, 0:CHALF],
            )

            st = stp.tile([128, 2, 6], f32)
            for sg in range(2):
                nc.vector.bn_stats(out=st[:nr, sg, :], in_=yc[:, sg, :])
            nc.vector.bn_aggr(out=mv_all[:nr, ic, :], in_=st[:nr])

        # rstd = 1/sqrt(var + eps), batched once (single ACT table switch)
        eps_t = singles.tile([128, 1], f32)
        nc.vector.memset(eps_t, EPS)
        rstd = singles.tile([128, NCHUNK], f32)
        nc.scalar.activation(
            out=rstd, in_=mv_all[:, :, 1], func=AF.Sqrt, bias=eps_t, scale=1.0
        )
        nc.vector.reciprocal(out=rstd, in_=rstd)

        for ic in range(NCHUNK):
            n0 = ic * PCH
            nr = min(PCH, RPC - n0)
            o_t = op.tile([128, C], f32)
            nc.vector.tensor_scalar(
                out=o_t[:nr],
                in0=y_all[:nr, ic, :],
                scalar1=mv_all[:nr, ic, 0:1],
                scalar2=rstd[:nr, ic : ic + 1],
                op0=ALU.subtract,
                op1=ALU.mult,
            )
            if apply_gamma_beta:
                nc.vector.tensor_mul(o_t[:nr], o_t[:nr], gamma_rep[:nr])
                nc.vector.tensor_add(o_t[:nr], o_t[:nr], beta_rep[:nr])
            nc.sync.dma_start(out=y_d[n0 : n0 + nr, :], in_=o_t[:nr])

    nc.compile()
    _CACHE[key] = nc
    return nc


def _host_prep(x, Wup, bup, Wdown, bdown, gamma, beta):
    """Build the per-core input maps (numpy only)."""
    bf = ml_dtypes.bfloat16
    xf = np.ascontiguousarray(np.asarray(x, np.float32).reshape(ROWS, C))
    Wup_ = np.asarray(Wup, np.float32).reshape(RANK, KTAPS)
    bup_ = np.asarray(bup, np.float32).reshape(RANK)
    wd_ = np.asarray(Wdown, np.float32).reshape(RANK)
    gamma_ = np.asarray(gamma, np.float32).reshape(C)
    beta_ = np.asarray(beta, np.float32).reshape(C)

    # Pre-windowed, transposable copy of x (bf16): window w holds
    # x_pad[:, 121w : 121w+127] plus a ones column for the bias row.
    xpad = np.zeros((ROWS, XPADC), np.float32)
    xpad[:, 3 : 3 + C] = xf
    cols = 121 * np.arange(NW)[:, None] + np.arange(WIN - 1)[None, :]
    xw = np.ones((ROWS, NW, WIN), np.float32)
    xw[:, :, : WIN - 1] = xpad[:, cols]
    xw = xw.astype(bf)

    # Banded conv weights [ci, rank_half, r4*121 + co]:
    #   band[co+k, rh, r4*121+co] = Wup[r, k];  band[127, ...] = bup[r]
    band = np.zeros((WIN, 2, 4 * WW), np.float32)
    co = np.arange(WW)
    for r in range(RANK):
        rh, r4 = divmod(r, 4)
        for k in range(KTAPS):
            band[co + k, rh, r4 * WW + co] = Wup_[r, k]
        band[WIN - 1, rh, r4 * WW : (r4 + 1) * WW] = bup_[r]
    band = band.astype(bf)

    # Scaled identities for the rank contraction.
    wdi = np.zeros((128, RANK, 128), np.float32)
    idx = np.arange(128)
    for r in range(RANK):
        wdi[idx, r, idx] = wd_[r]
    wdi = wdi.astype(bf)

    apply_gb = not (np.all(gamma_ == 1.0) and np.all(beta_ == 0.0))
    gb = np.stack([gamma_, beta_]).astype(np.float32)

    in_maps = []
    for i in range(NCORES):
        m = {
            "x": xf[i * RPC : (i + 1) * RPC],
            "xw": xw[i * RPC : (i + 1) * RPC],
            "band": band,
            "wdi": wdi,
        }
        if apply_gb:
            m["gb"] = gb
        in_maps.append(m)
    return in_maps, apply_gb


def kernel(x, Wup, bup, Wdown, bdown, gamma, beta):
    from concourse.bass_utils import run_bass_kernel_spmd

    in_maps, apply_gb = _host_prep(x, Wup, bup, Wdown, bdown, gamma, beta)
    nc = _build(apply_gb)
    res = run_bass_kernel_spmd(nc, in_maps, core_ids=list(range(NCORES)))
    y = np.concatenate([res.results[i]["y"] for i in range(NCORES)], axis=0)
    return np.ascontiguousarray(y.reshape(B, L, C).astype(np.float32))


# revision 8
# speedup vs baseline: 1.4027x; 1.4027x over previous
"""Trainium2 Bass kernel for nn_CrossVariableMixingConv.

Reference computation (per row of x, B*L rows, C=862 channels):
    h   = conv1d(x, Wup, k=7, pad=3) + bup      # (RANK=8 channels)
    g   = gelu(h)  (erf-exact)
    d   = sum_r Wdown[r] * g[r] + bdown
    y   = LayerNorm(x + d) * gamma + beta       # LN over C

Sharding: pure data parallelism — the B*L = 11520 rows are split into 8
shards of 1440 rows, one per NeuronCore; the ~70 conv params are
replicated.

Per-core kernel structure (rows on partitions, chunks of 128 rows):
  - The k=7 conv is a matmul with the *data* as the stationary operand:
    lhsT = xT window [128 c-taps, n rows] (DMA-transposed load of a
    pre-windowed bf16 copy of x, with a baked-in ones column), rhs = a
    banded weight matrix [128, 484] covering 4 ranks x 121 output
    columns; the ones row x bias row adds bup inside the same matmul.
  - Gelu runs on the Scalar engine directly out of PSUM (erf-exact Gelu
    LUT), writing bf16 into a per-rank G buffer in SBUF.
  - The rank contraction (Wdown) is 8 accumulating scaled-identity
    matmuls into PSUM, which leaves d already in row-major layout.
  - Residual + LayerNorm stats (bn_stats/bn_aggr) run on the Vector
    engine per chunk; rstd = 1/sqrt(var+eps) is batched once per core
    (single ACT table switch), then the normalization is applied with a
    fused (y - mu) * rstd tensor_scalar and results are DMA'd out.

bdown is dropped: LayerNorm is invariant to a constant shift per row.
gamma/beta are applied only when they are not the identity (they are
ones/zeros in this problem's setup_inputs).
"""

import sys

for _p in ("/opt/trn_rl_repo",):
    if _p not in sys.path:
        sys.path.insert(0, _p)

import numpy as np
import ml_dtypes

B, L, C = 16, 720, 862
RANK, KTAPS = 8, 7
NCORES = 8
ROWS = B * L                 # 11520
RPC = ROWS // NCORES         # 1440 rows per core
PCH = 128                    # rows per chunk (partition dim)
NCHUNK = (RPC + PCH - 1) // PCH   # 12 (11 full + 1 of 32)
WW = 121                     # conv output columns per window
NW = 8                       # windows: 8*121 = 968 >= 862
WIN = 128                    # window input columns (127 data + 1 ones)
GW = NW * WW                 # 968
XPADC = 976                  # padded x columns: 3 zeros + 862 + tail zeros
CHALF = C // 2               # 431
EPS = 1e-5

_CACHE: dict = {}


def _build(apply_gamma_beta: bool):
    """Build + compile the per-core Bass program. Cached per flag."""
    key = ("nc", apply_gamma_beta)
    if key in _CACHE:
        return _CACHE[key]

    from contextlib import ExitStack

    import concourse.bacc as bacc
    import concourse.bass as bass
    import concourse.tile as tile
    from concourse import mybir

    f32 = mybir.dt.float32
    bf16 = mybir.dt.bfloat16
    AF = mybir.ActivationFunctionType
    ALU = mybir.AluOpType

    nc = bacc.Bacc(
        "TRN2", target_bir_lowering=False, debug=False, num_devices=NCORES
    )

    x_d = nc.dram_tensor("x", [RPC, C], f32, kind="ExternalInput").ap()
    xw_d = nc.dram_tensor("xw", [RPC, NW, WIN], bf16, kind="ExternalInput").ap()
    band_d = nc.dram_tensor("band", [WIN, 2, 4 * WW], bf16, kind="ExternalInput").ap()
    wdi_d = nc.dram_tensor("wdi", [128, RANK, 128], bf16, kind="ExternalInput").ap()
    if apply_gamma_beta:
        gb_d = nc.dram_tensor("gb", [2, C], f32, kind="ExternalInput").ap()
    y_d = nc.dram_tensor("y", [RPC, C], f32, kind="ExternalOutput").ap()

    with tile.TileContext(nc) as tc, ExitStack() as ctx:
        singles = ctx.enter_context(tc.tile_pool(name="singles", bufs=1))
        xp = ctx.enter_context(tc.tile_pool(name="xin", bufs=2))
        gp = ctx.enter_context(tc.tile_pool(name="g", bufs=2))
        op = ctx.enter_context(tc.tile_pool(name="o", bufs=2))
        stp = ctx.enter_context(tc.tile_pool(name="st", bufs=2))
        pp = ctx.enter_context(tc.tile_pool(name="ps", bufs=2, space="PSUM"))

        band_t = singles.tile([WIN, 2, 4 * WW], bf16)
        nc.sync.dma_start(out=band_t, in_=band_d)
        wdi_t = singles.tile([128, RANK, 128], bf16)
        nc.sync.dma_start(out=wdi_t, in_=wdi_d)
        if apply_gamma_beta:
            gamma_rep = singles.tile([128, C], f32)
            beta_rep = singles.tile([128, C], f32)
            for rep, row in ((gamma_rep, 0), (beta_rep, 1)):
                src = bass.AP(
                    tensor=gb_d.tensor,
                    offset=gb_d.offset + row * C,
                    ap=[[0, 128], [1, C]],
                )
                nc.gpsimd.dma_start(out=rep, in_=src)

        y_all = singles.tile([128, NCHUNK, C], f32)
        mv_all = singles.tile([128, NCHUNK, 2], f32)
        # The ragged last chunk writes only 32 partitions; pre-fill so the
        # batched rstd (which reads the full tile) sees benign values.
        nc.vector.memset(mv_all, 1.0)

        # One big transposed load per conv window for the whole core
        # (small per-chunk transposes serialize on the xbar queue).
        # Separate tiles so each window's convs only wait on their own load.
        xtw = []
        for w in range(NW):
            t = singles.tile([WIN, RPC], bf16, tag=f"xtw{w}")
            nc.sync.dma_start_transpose(out=t, in_=xw_d[:, w, :])
            xtw.append(t)

        for ic in range(NCHUNK):
            n0 = ic * PCH
            nr = min(PCH, RPC - n0)

            x_t = xp.tile([128, C], f32)
            nc.sync.dma_start(out=x_t[:nr], in_=x_d[n0 : n0 + nr, :])

            G = gp.tile([128, RANK, GW], bf16)
            # G viewed as [p, w, rh, r4, c] for the gelu writes
            g_wview = G.rearrange("p (rh r4) (w c) -> p w rh r4 c", rh=2, c=WW)

            for g2 in range(4):  # groups of 2 windows -> 4 PSUM banks
                H = pp.tile([128, 4, 512], f32, tag="ps")
                for wg in range(2):
                    w = g2 * 2 + wg
                    for rh in range(2):
                        nc.tensor.matmul(
                            H[:nr, wg * 2 + rh, 0 : 4 * WW],
                            lhsT=xtw[w][:, n0 : n0 + nr],
                            rhs=band_t[:, rh, :],
                            start=True,
                            stop=True,
                        )
                h_view = H[:nr, :, 0 : 4 * WW].rearrange(
                    "p (wg rh) (r4 c) -> p wg rh r4 c", rh=2, c=WW
                )
                g_view = g_wview[:nr, g2 * 2 : g2 * 2 + 2]
                nc.scalar.activation(out=g_view, in_=h_view, func=AF.Gelu)

            # r-outer so each wdi_r is loaded once per chunk (2 matmuls/load)
            dT = pp.tile([128, 2, 512], f32, tag="ps")
            for r in range(RANK):
                for h in range(2):
                    nc.tensor.matmul(
                        dT[:nr, h, 0:CHALF],
                        lhsT=wdi_t[:nr, r, :nr],
                        rhs=G[:nr, r, h * CHALF : (h + 1) * CHALF],
                        start=(r == 0),
                        stop=(r == RANK - 1),
                    )

            yc = y_all[:nr, ic, :].rearrange("p (s c) -> p s c", s=2)
            nc.vector.tensor_add(
                out=yc,
                in0=x_t[:nr].rearrange("p (s c) -> p s c", s=2),
                in1=dT[:nr, :, 0:CHALF],
            )

            st = stp.tile([128, 2, 6], f32)
            for sg in range(2):
                nc.vector.bn_stats(out=st[:nr, sg, :], in_=yc[:, sg, :])
            nc.vector.bn_aggr(out=mv_all[:nr, ic, :], in_=st[:nr])

        # rstd = 1/sqrt(var + eps), batched once (single ACT table switch)
        eps_t = singles.tile([128, 1], f32)
        nc.vector.memset(eps_t, EPS)
        rstd = singles.tile([128, NCHUNK], f32)
        nc.scalar.activation(
            out=rstd, in_=mv_all[:, :, 1], func=AF.Sqrt, bias=eps_t, scale=1.0
        )
        nc.vector.reciprocal(out=rstd, in_=rstd)

        for ic in range(NCHUNK):
            n0 = ic * PCH
            nr = min(PCH, RPC - n0)
            o_t = op.tile([128, C], f32)
            nc.vector.tensor_scalar(
                out=o_t[:nr],
                in0=y_all[:nr, ic, :],
                scalar1=mv_all[:nr, ic, 0:1],
                scalar2=rstd[:nr, ic : ic + 1],
                op0=ALU.subtract,
                op1=ALU.mult,
            )
            if apply_gamma_beta:
                nc.vector.tensor_mul(o_t[:nr], o_t[:nr], gamma_rep[:nr])
                nc.vector.tensor_add(o_t[:nr], o_t[:nr], beta_rep[:nr])
            nc.sync.dma_start(out=y_d[n0 : n0 + nr, :], in_=o_t[:nr])

    nc.compile()
    _CACHE[key] = nc
    return nc


def _host_prep(x, Wup, bup, Wdown, bdown, gamma, beta):
    """Build the per-core input maps (numpy only)."""
    bf = ml_dtypes.bfloat16
    xf = np.ascontiguousarray(np.asarray(x, np.float32).reshape(ROWS, C))
    Wup_ = np.asarray(Wup, np.float32).reshape(RANK, KTAPS)
    bup_ = np.asarray(bup, np.float32).reshape(RANK)
    wd_ = np.asarray(Wdown, np.float32).reshape(RANK)
    gamma_ = np.asarray(gamma, np.float32).reshape(C)
    beta_ = np.asarray(beta, np.float32).reshape(C)

    # Pre-windowed, transposable copy of x (bf16): window w holds
    # x_pad[:, 121w : 121w+127] plus a ones column for the bias row.
    xpad = np.zeros((ROWS, XPADC), np.float32)
    xpad[:, 3 : 3 + C] = xf
    cols = 121 * np.arange(NW)[:, None] + np.arange(WIN - 1)[None, :]
    xw = np.ones((ROWS, NW, WIN), np.float32)
    xw[:, :, : WIN - 1] = xpad[:, cols]
    xw = xw.astype(bf)

    # Banded conv weights [ci, rank_half, r4*121 + co]:
    #   band[co+k, rh, r4*121+co] = Wup[r, k];  band[127, ...] = bup[r]
    band = np.zeros((WIN, 2, 4 * WW), np.float32)
    co = np.arange(WW)
    for r in range(RANK):
        rh, r4 = divmod(r, 4)
        for k in range(KTAPS):
            band[co + k, rh, r4 * WW + co] = Wup_[r, k]
        band[WIN - 1, rh, r4 * WW : (r4 + 1) * WW] = bup_[r]
    band = band.astype(bf)

    # Scaled identities for the rank contraction.
    wdi = np.zeros((128, RANK, 128), np.float32)
    idx = np.arange(128)
    for r in range(RANK):
        wdi[idx, r, idx] = wd_[r]
    wdi = wdi.astype(bf)

    apply_gb = not (np.all(gamma_ == 1.0) and np.all(beta_ == 0.0))
    gb = np.stack([gamma_, beta_]).astype(np.float32)

    in_maps = []
    for i in range(NCORES):
        m = {
            "x": xf[i * RPC : (i + 1) * RPC],
            "xw": xw[i * RPC : (i + 1) * RPC],
            "band": band,
            "wdi": wdi,
        }
        if apply_gb:
            m["gb"] = gb
        in_maps.append(m)
    return in_maps, apply_gb


def kernel(x, Wup, bup, Wdown, bdown, gamma, beta):
    from concourse.bass_utils import run_bass_kernel_spmd

    in_maps, apply_gb = _host_prep(x, Wup, bup, Wdown, bdown, gamma, beta)
    nc = _build(apply_gb)
    res = run_bass_kernel_spmd(nc, in_maps, core_ids=list(range(NCORES)))
    y = np.concatenate([res.results[i]["y"] for i in range(NCORES)], axis=0)
    return np.ascontiguousarray(y.reshape(B, L, C).astype(np.float32))
